# revision 8
# baseline (speedup 1.0000x reference)
"""BinLinear TRN2 kernel: out = x @ sign(weight).T + sign(bias).

Full shapes: x [8192, 4096] f32, weight [4096, 4096] f32, bias [4096] f32
-> out [8192, 4096] f32.

Sharding (8 NeuronCores): 2D grid, 4-way over tokens x 2-way over output
features. Each core computes out_c = x_c @ sign(w_c).T + sign(b_c) with
x_c [2048, 4096], w_c [2048, 4096], b_c [2048] -> out_c [2048, 2048].
The host only slices inputs and stitches the 4x2 output grid back together.

Per-core device program, organized so the PE never starves:
  - All input streaming is f32->f16 casting SWDGE DMA (gpsimd queue,
    ~390 GB/s) in [128, 2048] half-slab quanta; 4-deep stage pools keep
    cast -> sign/transpose stages overlapped across slabs.
  - Weights: ACT engine Sign (exact +-1, handles 0) in place on the f16
    stage, then XBAR transpose (sync queue) into resident wT[nb]
    [128, 32, 512] tiles (16 MB -- all weights stay in SBUF).
  - x: XBAR transpose into xT tiles [128, 32, 128] (one per 128-token
    slab, 4 rotating).
  - bias: sign(b) row; 4 seed matmuls replicate it into brep [128, 2048]
    f16; copy-out does osb16 = psum + brep on DVE (no per-tile bias
    matmul on the PE).
  - PE: 2048 fp16 matmuls (K=128, N=512), ~220 ns each, two phases:
    Phase 1 feature-block-major over token slabs 0-3 while weights
    stream (w blocks 2-3 are issued between sections so every engine
    queue's program order matches execution order); Phase 2 token-major
    over slabs 4-15, kt-outer/feature-inner so four PSUM banks
    accumulate per stationary xT load.
  - Output is written as fp16 (halves DMA traffic); the host upcasts.
"""

import sys

if "/opt/trn_rl_repo" not in sys.path:
    sys.path.insert(0, "/opt/trn_rl_repo")

from contextlib import ExitStack

import numpy as np

import concourse.bass as bass
import concourse.mybir as mybir
import concourse.tile as tile
from concourse import bacc
from concourse.bass_utils import run_bass_kernel_spmd
from concourse.tile_rust import add_dep_helper

N_TOK, D_IN, D_OUT = 8192, 4096, 4096
TOK_WAYS, OUT_WAYS = 4, 2
N_CORES = TOK_WAYS * OUT_WAYS
TOK_SH = N_TOK // TOK_WAYS    # 2048 tokens per core
OUT_SH = D_OUT // OUT_WAYS    # 2048 out features per core

P = 128
KT = D_IN // P                # 32 contraction subtiles
NFREE = 512                   # PSUM free dim per matmul
NB = OUT_SH // NFREE          # 4 feature blocks
NWS = OUT_SH // P             # 16 weight slabs (128 feats each)
HK = D_IN // 2                # half-slab K split (2048)
NXS = TOK_SH // P             # 16 x slabs (128 tokens each)

F16 = mybir.dt.float16
F32 = mybir.dt.float32


def _build():
    nc = bacc.Bacc("TRN2", target_bir_lowering=False, debug=False,
                   num_devices=N_CORES)
    x = nc.dram_tensor("x", [TOK_SH, D_IN], F32, kind="ExternalInput")
    w = nc.dram_tensor("w", [OUT_SH, D_IN], F32, kind="ExternalInput")
    b = nc.dram_tensor("b", [1, OUT_SH], F32, kind="ExternalInput")
    out = nc.dram_tensor("out", [TOK_SH, OUT_SH], F16, kind="ExternalOutput")

    with ExitStack() as ctx:
        tc = ctx.enter_context(tile.TileContext(nc))
        wsgp = ctx.enter_context(tc.tile_pool(name="wsgp", bufs=4))
        wTp = ctx.enter_context(tc.tile_pool(name="wTp", bufs=NB))
        xstp = ctx.enter_context(tc.tile_pool(name="xstp", bufs=4))
        xTp = ctx.enter_context(tc.tile_pool(name="xTp", bufs=4))
        osbp = ctx.enter_context(tc.tile_pool(name="osbp", bufs=4))
        constp = ctx.enter_context(tc.tile_pool(name="constp", bufs=1))
        mmps = ctx.enter_context(tc.tile_pool(name="mmps", bufs=8, space="PSUM"))

        # per-queue nosync chains pin DMA issue order
        last_q = {}

        def chain(inst, q):
            if q in last_q:
                add_dep_helper(inst.ins, last_q[q].ins, sync=False,
                               reason=f"{q} order")
            last_q[q] = inst
            return inst

        # ---- bias: brow = sign(b) f16, replicated into brep f16 [128, 2048]
        ones = constp.tile([1, P], F16)
        nc.gpsimd.memset(ones[:], 1.0)
        brow = constp.tile([1, OUT_SH], F16)
        chain(nc.gpsimd.dma_start(brow[:], b[:]), "in")   # casting DMA
        nc.scalar.sign(brow[:], brow[:])
        brep = constp.tile([P, OUT_SH], F16)
        for nb in range(NB):
            ps = mmps.tile([P, NFREE], F32, tag="mm", name=f"bps{nb}")
            nc.tensor.matmul(ps[:], ones[:], brow[:, nb * NFREE:(nb + 1) * NFREE],
                             start=True, stop=True)
            nc.vector.tensor_copy(brep[:, nb * NFREE:(nb + 1) * NFREE], ps[:])

        # ---- weight stream: slab j (128 feats), half h (2048 K).
        # SWDGE f32->f16 cast, ACT Sign in place, XBAR into wT.
        wT = [wTp.tile([P, KT, NFREE], F16, tag="wT", name=f"wT{i}")
              for i in range(NB)]

        def w_half(j, h):
            sg = wsgp.tile([P, HK], F16, tag="wsg", name=f"wsg{j}_{h}")
            chain(nc.gpsimd.dma_start(sg[:], w[j * P:(j + 1) * P, h * HK:(h + 1) * HK]), "in")
            nc.scalar.sign(sg[:], sg[:])
            nb, jj = j // NB, j % NB
            dst = wT[nb][:, h * (KT // 2):(h + 1) * (KT // 2), jj * P:(jj + 1) * P]
            chain(nc.sync.dma_start_transpose(dst, sg[:]), "t")

        def w_slab(j):
            w_half(j, 0), w_half(j, 1)

        # ---- x stream: slab t (128 tokens), half h (2048 K)
        xT = [None] * NXS

        def x_half(t, h):
            if xT[t] is None:
                xT[t] = xTp.tile([P, KT, P], F16, tag="xT", name=f"xT{t}")
            st = xstp.tile([P, HK], F16, tag="xst", name=f"xst{t}_{h}")
            chain(nc.gpsimd.dma_start(st[:], x[t * P:(t + 1) * P, h * HK:(h + 1) * HK]), "in")
            dst = xT[t][:, h * (KT // 2):(h + 1) * (KT // 2), :]
            chain(nc.sync.dma_start_transpose(dst, st[:]), "t")

        def x_slab(t):
            x_half(t, 0), x_half(t, 1)

        # ---- issue order: w block 0, x slabs 0-3, w block 1; blocks 2-3
        # are issued between phase-1 sections.
        for j in (0, 1, 2, 3):
            w_slab(j)
        for t in (0, 1, 2, 3):
            x_slab(t)
        for j in (4, 5, 6, 7):
            w_slab(j)

        def copy_out(psum, row0, nb):
            osb = osbp.tile([P, NFREE], F16, tag="osb", name="osb")
            nc.vector.tensor_tensor(osb[:], psum[:],
                                    brep[:, nb * NFREE:(nb + 1) * NFREE],
                                    mybir.AluOpType.add)
            nc.scalar.dma_start(
                out[row0:row0 + P, nb * NFREE:(nb + 1) * NFREE], osb[:])

        # ---- Phase 1: slabs 0-3, feature-block-major; slab pairs share a
        # PSUM bank pair, kt-outer so banks interleave.
        def p1_section(nb):
            for cp in range(2):
                cc = (2 * cp, 2 * cp + 1)
                psums = [mmps.tile([P, NFREE], F32, tag="mm", name=f"p1_{nb}_{c}")
                         for c in cc]
                for kt in range(KT):
                    for i, c in enumerate(cc):
                        nc.tensor.matmul(
                            psums[i][:], xT[c][:, kt, :], wT[nb][:, kt, :],
                            start=(kt == 0), stop=(kt == KT - 1),
                        )
                for i, c in enumerate(cc):
                    copy_out(psums[i], c * P, nb)

        p1_section(0)
        p1_section(1)
        # w blocks 2-3 stream while nb=2,3 sections run
        for j in (8, 9, 10, 11):
            w_slab(j)
        for j in (12, 13, 14, 15):
            w_slab(j)
        p1_section(2)
        p1_section(3)

        # ---- stream in slabs 4-15 (xT pool rotation paces the casts and
        # transposes behind consumption)
        for t in range(4, NXS):
            x_slab(t)

        # ---- Phase 2: slabs 4-15, token-major; kt-outer/feature-inner so
        # four PSUM banks accumulate per stationary xT load.
        for c in range(4, NXS):
            psums = [mmps.tile([P, NFREE], F32, tag="mm", name=f"p2_{c}_{nb}")
                     for nb in range(NB)]
            for kt in range(KT):
                lhsT = xT[c][:, kt, :]
                for nb in range(NB):
                    nc.tensor.matmul(
                        psums[nb][:], lhsT, wT[nb][:, kt, :],
                        start=(kt == 0), stop=(kt == KT - 1),
                    )
            for nb in range(NB):
                copy_out(psums[nb], c * P, nb)

    nc.finalize()
    return nc


_cache = {}


def _get_nc(exact_sign: bool = False):
    # single program: ACT Sign handles zero weights/bias exactly
    if "nc" not in _cache:
        _cache["nc"] = _build()
    return _cache["nc"]


def kernel(x: np.ndarray, weight: np.ndarray, bias: np.ndarray) -> np.ndarray:
    x = np.ascontiguousarray(np.asarray(x, dtype=np.float32))
    weight = np.ascontiguousarray(np.asarray(weight, dtype=np.float32))
    bias = np.ascontiguousarray(np.asarray(bias, dtype=np.float32))
    assert x.shape == (N_TOK, D_IN) and weight.shape == (D_OUT, D_IN)

    nc = _get_nc()

    in_maps = []
    for tg in range(TOK_WAYS):
        for og in range(OUT_WAYS):
            in_maps.append({
                "x": np.ascontiguousarray(x[tg * TOK_SH:(tg + 1) * TOK_SH, :]),
                "w": np.ascontiguousarray(weight[og * OUT_SH:(og + 1) * OUT_SH, :]),
                "b": np.ascontiguousarray(
                    bias[og * OUT_SH:(og + 1) * OUT_SH].reshape(1, OUT_SH)),
            })

    res = run_bass_kernel_spmd(nc, in_maps, list(range(N_CORES)))

    out = np.empty((N_TOK, D_OUT), dtype=np.float32)
    c = 0
    for tg in range(TOK_WAYS):
        for og in range(OUT_WAYS):
            out[tg * TOK_SH:(tg + 1) * TOK_SH, og * OUT_SH:(og + 1) * OUT_SH] = \
                res.results[c]["out"].astype(np.float32)
            c += 1
    return out
